# revision 29
# baseline (speedup 1.0000x reference)
"""Bilateral filter (7x7, sigma_color=0.1) Trainium2 Bass kernel — Design S2.

Strategy (vs. the strip-layout baseline):
  - Shard H across 8 cores (90 rows each), full 1280 width. Flat row layout:
    SBUF tile IM[102, 3, 1292] fp16 = rows -6..95, channel-major, cols -6..1285.
  - Weight-field symmetry: W_{dy,dx}[u] == W_{6-dy,6-dx}[u + (dy-3,dx-3)], so
    only 24 shift-pairs (+ the free center shift) need D/exp. Per pair j=(ay,ax):
      SB = IM[shifted] - IM[center]            (DVE, [96,3,646] fp16, 2x mode)
      SQ = SB^2                                (ACT Square)
      D  = SQ0+SQ1+SQ2                         (Pool, 2 fused scalar_tensor_tensor)
      F  = exp(-50*D + b_j)                    (ACT, b_j = ln(norm*g_j))
      H  = F * IM[shifted]   (num term j)      (DVE, [90,3,640])
      G  = F * IM[center]    (num term j')     (DVE 2ch + Pool 1ch, [96,3,646])
  - Accumulation on PE into PSUM[90, 4, 640] fp32 (num0..2, den), 2 x-passes
    of 640 cols (PSUM capacity). All matmuls use ONE stationary identity lhsT;
    row/col mirror shifts are rhs partition-offset / column-offset views:
      num += H                      (lhsT=Id, rhs=H)
      num += G[r-ay, x-ax]          (lhsT=Id, rhs=G partitions 3-ay.., cols -ax)
      den += F[r,x] + F[r-ay,x-ax]  (same, rhs=F views)
      center: num += c0*IM, den += c0   (c0 = norm*g_33; tiny extra matmuls)
  - Finalize per pass: rec = 1/den (DVE), out = num*rec (DVE fp32), DMA out.
  - HBM traffic ~2.2 MB/core (vs 38.7 MB for the host-expanded strip design).
"""

import math

import numpy as np

import concourse.bass as bass
import concourse.bacc as bacc
import concourse.mybir as mybir
from concourse.tile import TileContext

F16 = np.float16
F32 = np.float32

H, W, C = 720, 1280, 3
K = 7
PAD = 3
SIGMA_COLOR = 0.1
NORM_COLOR = 1.0 / (2.0 * math.pi * SIGMA_COLOR**2)
EXP_SCALE = -1.0 / (2.0 * SIGMA_COLOR**2)  # -50.0

N_CORES = 8
RPC = H // N_CORES           # 90 output rows per core
XW = 640                     # pass width (PSUM capacity: 4 * 640 fp32 = 5 banks)
NPASS = W // XW              # 2
EXT = 12                     # input halo rows (2*2*PAD)
PW = W + 12                  # padded width 1292

# shift pairs: (ay, ax) with ay<0, or ay==0 and ax<0  (24 of 49; center free).
# Ordered ay=0 first: those pairs only need the IMS[0] copy, so compute can
# begin before the other shifted image copies finish loading.
PAIRS = [(dy - 3, dx - 3) for dy in (3, 2, 1, 0) for dx in range(K)
         if (dy < 3) or (dy == 3 and dx < 3)]
assert len(PAIRS) == 24

# psum bank-aligned chunks per quantity: (col_start, ncols) within [0, 640),
# global col = 640*q + cs must not cross a 512-col (2KB) bank boundary
CHUNKS = {
    0: [(0, 512), (512, 128)],
    1: [(0, 384), (384, 256)],
    2: [(0, 256), (256, 384)],
    3: [(0, 128), (128, 512)],
}


def _alu(name):
    return getattr(mybir.AluOpType, name)


def build_nc(rows: int = RPC):
    """One core: rows output rows (+12 halo), full width, 2 passes of XW."""
    dt = mybir.dt
    nc = bacc.Bacc("TRN2", debug=False)

    np_ = rows + EXT            # input tile partitions (102)
    fe = rows + 2 * PAD         # F/G extension rows (96)

    IM = nc.dram_tensor("IM", [np_, C, PW], dt.float16, kind="ExternalInput")
    WT = nc.dram_tensor("WT", [fe, 540 + fe], dt.float16, kind="ExternalInput")
    BI = nc.dram_tensor("BI", [128, 24], dt.float32, kind="ExternalInput")
    ON = nc.dram_tensor("ON", [1, XW], dt.float16, kind="ExternalInput")
    OUT = nc.dram_tensor("OUT", [C, rows, W], dt.float32, kind="ExternalOutput")

    with TileContext(nc) as tc:
        with (
            tc.tile_pool(name="persist", bufs=1) as pp,
            tc.tile_pool(name="sub", bufs=5) as psub,
            tc.tile_pool(name="sq", bufs=5) as psq,
            tc.tile_pool(name="ff", bufs=6) as pf,
            tc.tile_pool(name="hh", bufs=5) as ph,
            tc.tile_pool(name="gg", bufs=5) as pg,
            tc.tile_pool(name="fin", bufs=2) as pfin,
            tc.tile_pool(name="psum", bufs=1, space="PSUM") as pps,
            tc.tile_pool(name="psumda", bufs=2, space="PSUM") as ppsda,
            tc.tile_pool(name="psumdb", bufs=1, space="PSUM") as ppsdb,
        ):
            # SBUF compute ops require base partition 0 (or 32/64/96), so
            # row shifts can't be partition-offset views. Instead build 7
            # row-shifted SBUF copies via DMA (no partition restriction):
            # IMS[d][p, c, xi] = Ipad[row p-3+d, col xi],  p=0..95
            # small constant loads first — the first matmuls/exp need them
            wt = pp.tile([fe, 540 + fe], dt.float16, tag="wt")
            nc.sync.dma_start(wt[:, :], WT[:, :])
            bi = pp.tile([128, 24], dt.float32, tag="bi")
            nc.sync.dma_start(bi[:, :], BI[:, :])
            ones = pp.tile([1, XW], dt.float16, tag="ones")
            nc.sync.dma_start(ones[:, :], ON[:, :])
            IMS = {}
            # load order matches pair consumption order (ay = 0 first), and
            # x-lower halves first: pass-0 compute only reads cols < 656, so
            # it can start while the upper halves are still in flight
            half = 656
            for d in (0, -1, -2, -3, 1, 2, 3):
                t = pp.tile([fe, C, PW], dt.float16, tag=f"ims{d}")
                if d == 0:
                    # the first pairs (ay=0) gate on this tile alone: chunk
                    # it across DMA queues (one dma_start rides one queue)
                    for p0 in range(0, fe, 24):
                        p1 = min(p0 + 24, fe)
                        nc.sync.dma_start(
                            t[p0:p1, :, 0:half],
                            IM[3 + p0:3 + p1, :, 0:half])
                else:
                    nc.sync.dma_start(
                        t[:, :, 0:half], IM[3 + d:3 + d + fe, :, 0:half])
                IMS[d] = t
            for d in (0, -1, -2, -3, 1, 2, 3):
                nc.sync.dma_start(
                    IMS[d][:, :, half:PW], IM[3 + d:3 + d + fe, :, half:PW])

            # lhsT views, all base partition 0:
            # SH[k][p, r] = 1 iff p == r + k (k=3..6); CID = c0*SH[3]; OC = c0 row
            SH = {k: wt[:, (k - 3) * rows:(k - 2) * rows] for k in range(3, 7)}
            CID = wt[:, 4 * rows:5 * rows]
            OC = wt[0:1, 5 * rows:6 * rows]
            I96 = wt[:, 6 * rows:6 * rows + fe]   # identity [fe, fe]

            # software-pipelined emission: {sub, sq} of unit k+1 is emitted
            # before {D-mm, exp, H, G, mms} of unit k so each engine's FIFO
            # has ready work ahead of cross-engine waits.
            pstate = {}   # p_i -> (ps tile, started banks set)
            sbq, sqq = {}, {}

            def stage_sub(p_i, jp):
                # DVE subtract only — emitted one unit ahead so the DVE FIFO
                # has ready work while H(k) waits on exp(k) (ACT/PE order is
                # left untouched; reordering ACT measured worse)
                x0 = p_i * XW
                ay, ax = PAIRS[jp]
                sb = psub.tile([fe, C, 646], dt.float16, tag="sb", name="sb")
                nc.vector.tensor_tensor(
                    sb[:, :, :],
                    IMS[ay][:, :, x0 + 3 + ax:x0 + 649 + ax],
                    IMS[0][:, :, x0 + 3:x0 + 649],
                    _alu("subtract"),
                )
                sbq[(p_i, jp)] = sb

            def stage_sq(p_i, jp):
                sb = sbq[(p_i, jp)]
                sq = psq.tile([fe, C, 646], dt.float16, tag="sq", name="sq")
                nc.scalar.activation(
                    sq[:, :, :], sb[:, :, :],
                    mybir.ActivationFunctionType.Square,
                )
                sqq[(p_i, jp)] = sq

            ffq = {}

            def stage_b1(p_i, jp):
                sq = sqq.pop((p_i, jp))
                del sbq[(p_i, jp)]
                # channel sum on PE: D-psum += sq_c via identity matmuls
                # (GpSimd contends with DVE for SBUF; PE does this free).
                # D is split: a double-buffered 512-col tile plus a single
                # 134-col tile, so the next pair's D-matmuls don't wait on
                # this pair's exp (two cheap exps instead of one).
                psda = ppsda.tile([fe, 512], dt.float32, tag="psda", name="psda")
                psdb = ppsdb.tile([fe, 512], dt.float32, tag="psdb", name="psdb")
                for c in range(C):
                    nc.tensor.matmul(
                        psda[:, :], I96, sq[:, c, 0:512],
                        start=(c == 0), stop=(c == C - 1),
                    )
                    nc.tensor.matmul(
                        psdb[:, 0:134], I96, sq[:, c, 512:646],
                        start=(c == 0), stop=(c == C - 1),
                    )
                ff = pf.tile([fe, 646], dt.float16, tag="ff", name="ff")
                nc.scalar.activation(
                    ff[:, 0:512], psda[:, :],
                    mybir.ActivationFunctionType.Exp,
                    bias=bi[0:fe, jp:jp + 1], scale=float(EXP_SCALE),
                )
                nc.scalar.activation(
                    ff[:, 512:646], psdb[:, 0:134],
                    mybir.ActivationFunctionType.Exp,
                    bias=bi[0:fe, jp:jp + 1], scale=float(EXP_SCALE),
                )
                ffq[(p_i, jp)] = ff

            def stage_b2(p_i, jp):
                x0 = p_i * XW
                ay, ax = PAIRS[jp]
                if p_i not in pstate:
                    pstate[p_i] = (
                        pps.tile([rows, 4, XW], dt.float32, tag="ps",
                                 name=f"ps{p_i}"),
                        set(),
                    )
                ps, started = pstate[p_i]

                def mm(q, lhsT, rhs_of):
                    for ci, (cs, cn) in enumerate(CHUNKS[q]):
                        bank = (2560 * q + 4 * cs) // 2048
                        st = bank not in started
                        started.add(bank)
                        nc.tensor.matmul(
                            ps[:, q, cs:cs + cn], lhsT, rhs_of(cs, cn),
                            start=st, stop=False,
                        )

                ff = ffq.pop((p_i, jp))
                # products
                hh = ph.tile([fe, C, XW], dt.float16, tag="hh", name="hh")
                nc.vector.tensor_tensor(
                    hh[:, :, :],
                    ff[:, None, 3:3 + XW].to_broadcast([fe, C, XW]),
                    IMS[ay][:, :, x0 + 6 + ax:x0 + 646 + ax],
                    _alu("mult"),
                )
                gg = pg.tile([fe, C, 646], dt.float16, tag="gg", name="gg")
                nc.vector.tensor_tensor(
                    gg[:, :, :],
                    ff[:, None, :].to_broadcast([fe, C, 646]),
                    IMS[0][:, :, x0 + 3:x0 + 649],
                    _alu("mult"),
                )
                # accumulate: H-term (SH[3]) and G-term (SH[3-ay], col
                # shift via rhs view)
                ks = 3 - ay
                for q in range(C):
                    mm(q, SH[3], lambda cs, cn, q=q: hh[:, q, cs:cs + cn])
                mm(3, SH[3], lambda cs, cn: ff[:, 3 + cs:3 + cs + cn])
                for q in range(C):
                    mm(q, SH[ks], lambda cs, cn, q=q: gg[
                        :, q, 3 - ax + cs:3 - ax + cs + cn])
                mm(3, SH[ks], lambda cs, cn: ff[
                    :, 3 - ax + cs:3 - ax + cs + cn])

            def finish_pass(p_i):
                x0 = p_i * XW
                ps, started = pstate.pop(p_i)
                # center shift: num += c0*I, den += c0. den (q=3) first so
                # the reciprocal can start while num-center matmuls run.
                # Last matmul touching each bank closes its group.
                cen = [(3, cs, cn) for cs, cn in CHUNKS[3]]
                cen += [(q, cs, cn) for q in range(C) for cs, cn in CHUNKS[q]]
                banks = [(2560 * q + 4 * cs) // 2048 for q, cs, cn in cen]
                for i, (q, cs, cn) in enumerate(cen):
                    rhs = (ones[:, cs:cs + cn] if q == 3
                           else IMS[0][:, q, x0 + 6 + cs:x0 + 6 + cs + cn])
                    nc.tensor.matmul(
                        ps[:, q, cs:cs + cn], OC if q == 3 else CID, rhs,
                        start=False, stop=banks[i] not in banks[i + 1:],
                    )
                # finalize (den is well inside fp32 normal range); per-channel
                # so each output DMA starts as soon as possible
                rec = pfin.tile([rows, XW], dt.float32, tag="rec", name="rec")
                nc.vector.reciprocal_approx_fast(rec[:, :], ps[:, 3, :])
                ot = pfin.tile([rows, C, XW], dt.float32, tag="ot", name="ot")
                for c in range(C):
                    nc.vector.tensor_tensor(
                        ot[:, c, :], ps[:, c, :], rec[:, :], _alu("mult"),
                    )
                    nc.sync.dma_start(OUT[c, :, x0:x0 + XW], ot[:, c, :])

            # in-order emission except the DVE sub, which runs one unit
            # ahead (emitted between ff(k) and H(k)) so DVE never idles on
            # the exp(k) -> H(k) dependency
            units = [(p, j) for p in range(NPASS) for j in range(len(PAIRS))]
            stage_sub(*units[0])
            for i, u in enumerate(units):
                stage_sq(*u)
                stage_b1(*u)
                if i + 1 < len(units):
                    stage_sub(*units[i + 1])
                stage_b2(*u)
                if u[1] == len(PAIRS) - 1:
                    finish_pass(u[0])

    nc.compile()
    return nc


def host_prepare(I: np.ndarray, gw49: np.ndarray):
    """I: (1, C, H, W) fp32, gw49: (49,). Returns per-core input maps."""
    _, c_, him, wim = I.shape
    rows = him // N_CORES
    np_ = rows + EXT

    Ip = np.zeros((C, him + EXT, wim + EXT), dtype=F16)
    Ip[:, 6:6 + him, 6:6 + wim] = I[0].astype(F16)

    g7 = gw49.reshape(K, K).astype(np.float64)
    c0 = float(NORM_COLOR * g7[3, 3])

    fe = rows + 2 * PAD
    wt = np.zeros((fe, 540 + fe), dtype=F16)
    idx = np.arange(rows)
    for k in range(3, 7):                  # SH[k]: p == r+k
        wt[idx + k, (k - 3) * rows + idx] = 1.0
    wt[idx + 3, 4 * rows + idx] = c0       # CID = c0 * SH[3]
    wt[0, 5 * rows:6 * rows] = c0          # OC row
    ide = np.arange(fe)
    wt[ide, 6 * rows + ide] = 1.0          # I96 identity

    bi = np.zeros((128, 24), dtype=F32)
    for jp, (ay, ax) in enumerate(PAIRS):
        bi[:, jp] = math.log(NORM_COLOR * g7[ay + 3, ax + 3])

    on = np.ones((1, XW), dtype=F16)

    in_maps = []
    for i in range(N_CORES):
        sh = Ip[:, rows * i:rows * i + np_, :]           # [C, np_, PW]
        imt = np.ascontiguousarray(sh.transpose(1, 0, 2))  # [np_, C, PW]
        in_maps.append({"IM": imt, "WT": wt, "BI": bi, "ON": on})
    return in_maps, rows


def assemble(results, him, wim, rows):
    out = np.empty((1, C, him, wim), dtype=F32)
    for i in range(N_CORES):
        out[0, :, rows * i:rows * i + rows, :] = results[i]["OUT"]
    return out


def _numpy_fallback(I, g):
    n, c, h, w = I.shape
    Ipad = np.zeros((n, c, h + 2 * PAD, w + 2 * PAD), dtype=np.float64)
    Ipad[:, :, PAD:PAD + h, PAD:PAD + w] = I
    num = np.zeros((n, c, h, w), dtype=np.float64)
    den = np.zeros((n, h, w), dtype=np.float64)
    g64 = g.astype(np.float64)
    for j in range(K * K):
        dy, dx = j // K, j % K
        S = Ipad[:, :, dy:dy + h, dx:dx + w]
        D = ((S - I.astype(np.float64)) ** 2).sum(axis=1)
        wgt = np.exp(EXP_SCALE * D) * NORM_COLOR * g64[:, j]
        num += wgt[:, None] * S
        den += wgt
    return (num / den[:, None]).astype(F32)


_CACHE = {}
TRACE = False
LAST_EXEC_NS = None
_LDW_PATCHED = False


def _enable_ldw_prune():
    """Drop duplicate LDWEIGHTS of the same stationary lhsT (PE weights
    persist across matmuls)."""
    global _LDW_PATCHED
    if _LDW_PATCHED:
        return
    import json as _json
    import concourse.bass_utils as _bu

    _orig = _bu.compile_bir_kernel

    def _prune(bir_json):
        js = _json.loads(bir_json)
        for fn in js.get("functions", []):
            for blk in fn.get("blocks", []):
                insts = blk.get("instructions", [])
                out = []
                last_ldw = None
                for inst in insts:
                    if inst.get("opcode") == "Ldweights":
                        si = inst.get("sync_info") or {}
                        key = _json.dumps(inst.get("ins"), sort_keys=True)
                        if (
                            last_ldw == key
                            and not si.get("on_wait")
                            and not si.get("on_update")
                        ):
                            continue
                        last_ldw = key
                    out.append(inst)
                blk["instructions"] = out
        return _json.dumps(js).encode()

    def _patched(bir_json, tmpdir, neff_name="file.neff"):
        try:
            bir_json = _prune(bir_json)
        except Exception:
            pass
        return _orig(bir_json, tmpdir, neff_name=neff_name)

    _bu.compile_bir_kernel = _patched
    try:
        import concourse.bass2jax as _b2j

        if getattr(_b2j, "compile_bir_kernel", None) is not None:
            _b2j.compile_bir_kernel = _patched
    except Exception:
        pass
    _LDW_PATCHED = True


def kernel(I: np.ndarray, g: np.ndarray) -> np.ndarray:
    global LAST_EXEC_NS
    I = np.asarray(I, dtype=F32)
    g = np.asarray(g)

    gw49 = np.asarray(g[0, :, 0, 0], dtype=F32)
    g7 = gw49.reshape(K, K)
    spatially_const = np.array_equal(
        np.asarray(g), np.broadcast_to(np.asarray(g)[:, :, :1, :1], g.shape)
    )
    symmetric = np.allclose(g7, g7[::-1, ::-1], rtol=1e-6, atol=0)
    if not (spatially_const and symmetric):
        return _numpy_fallback(I, g)

    from concourse.bass_utils import run_bass_kernel_spmd

    import os as _os
    if _os.environ.get("BASS_LDW_PRUNE", "1") == "1":
        _enable_ldw_prune()

    in_maps, rows = host_prepare(I, gw49)
    key = rows
    if key not in _CACHE:
        _CACHE[key] = build_nc(rows)
    nc = _CACHE[key]
    res = run_bass_kernel_spmd(
        nc, in_maps, core_ids=list(range(N_CORES)), trace=TRACE
    )
    LAST_EXEC_NS = res.exec_time_ns
    return assemble(res.results, I.shape[2], I.shape[3], rows)


if __name__ == "__main__":
    # single-core CoreSim numeric check vs numpy on the core-0 slice
    import concourse.bass_interp as bass_interp

    rng = np.random.default_rng(0)
    I = rng.random((1, C, H, W), dtype=F32)
    gw49 = np.exp(
        -(np.add.outer(np.arange(-3.0, 4) ** 2, np.arange(-3.0, 4) ** 2)) / 50.0
    ).reshape(-1) * (2 * math.pi * 25.0)
    g = np.tile(gw49.reshape(1, K * K, 1, 1), (1, 1, H, W)).astype(F32)

    in_maps, rows = host_prepare(I, gw49.astype(F32))
    nc = build_nc(rows)
    sim = bass_interp.CoreSim(nc)
    for k, v in in_maps[0].items():
        sim.tensor(k)[:] = v
    sim.simulate()
    got = np.array(sim.tensor("OUT"))

    exp_full = _numpy_fallback(I, g)
    exp0 = exp_full[0, :, 0:rows, :]
    err = np.abs(got - exp0)
    print("sim err max:", err.max(), "rel:", err.max() / np.abs(exp0).max())
